# revision 11
# baseline (speedup 1.0000x reference)
"""Debayer3x3 Trainium2 Bass kernel.

Reference computation (per image, H=W=2048, f32):
  xpad = reflect-pad(x, 1)
  cross = 0.25*(up+down+left+right), diag = 0.25*(4 diagonals),
  hz = 0.5*(left+right), vt = 0.5*(up+down)
  R = [[x, hz], [vt, diag]]  (2x2 row/col parity pattern)
  G = [[cross, x], [x, cross]]
  B = [[diag, vt], [hz, x]]

Strategy: pure data parallel, 1 image per NeuronCore (batch 8 / 8 cores).
Per core, tiles put K=16 consecutive image rows in each SBUF partition so all
vertical neighbor access is a free-dim shift; one band of 128 partitions
covers the whole image, so the halo re-read is only 2/K = 12.5%.

All intermediates are bf16 (output tolerance is 2e-2; bf16 rounding is
~6e-3 worst case here and every scale is a power of two, hence exact):
  - loads cast f32->bf16 in-flight on the SWDGE (gpsimd) path, halving both
    SBUF footprint and DVE time (bf16 tensor_tensor runs in 2x perf mode),
  - X is prescaled by 0.25 in place (bf16 tensor_scalar, 4x mode) so every
    final op needs only a free ACT scale or a plain add,
  - quadrant ops write an f32 output tile directly (engine output cast).

Work planes per compute chunk (X holds 0.25*x):
  Hq[jx,c] = X[jx,c] + X[jx,c+2]   (= 0.25*(left+right) at row jx, col c)
  Vq[j,c]  = X[j,c+1] + X[j+2,c+1] (= 0.25*(up+down) at out row j, col c)
Each output quadrant is one scaled ACT copy (hz=2*Hq, vt=2*Vq, x=4*X) on the
scalar engine or one strided bf16 tensor_add (diag=Hq[j]+Hq[j+2],
cross=Vq[j]+Hq[j+1]) on the vector engine, written into an interleaved
[p, 3ch, K, WCC] f32 output tile.

DMA: loads (incl. halos) ride the SWDGE ring (nc.gpsimd, needed for the
dtype cast); f32 stores alternate between the two HWDGE rings (nc.scalar /
nc.sync) so three DMA paths drain concurrently.
"""

from contextlib import ExitStack

import numpy as np

H, W = 2048, 2048
K = 16           # image rows per partition (one 128-partition band = image)
WCL = 256        # load chunk width
WCC = 128        # compute/store chunk width
N_CORES = 8

_compiled = {}


def _build(nc_mod, tile_mod, mybir, h, w, k, wcl, wcc, bacc_mod=None):
    """Emit the debayer program for one core into a fresh Bass object."""
    bass = nc_mod
    p = 128
    nl = w // wcl          # load chunks per row-band
    ncc = wcl // wcc       # compute chunks per load chunk
    assert h == p * k and w == nl * wcl and wcl == ncc * wcc
    assert k % 2 == 0 and wcc % 2 == 0
    dt = mybir.dt.float32
    db = mybir.dt.bfloat16
    Copy = mybir.ActivationFunctionType.Copy

    nc = bass.Bass() if bacc_mod is None else bacc_mod.Bacc("TRN2")
    x = nc.dram_tensor("x", [h, w], dt, kind="ExternalInput")
    out = nc.dram_tensor("out", [3, h, w], dt, kind="ExternalOutput")
    outr = out.rearrange("c (p k) w -> p c k w", p=p, k=k)

    # row slices into the output tile (j) and into X/Hq (jx = j+1 is the same
    # image row; X/Hq rows 0..k+1 carry the halo)
    je, jo = slice(0, k, 2), slice(1, k, 2)
    jxe, jxo = slice(1, k + 1, 2), slice(2, k + 2, 2)
    jm_e, jp_e = slice(0, k, 2), slice(2, k + 2, 2)      # j, j+2 for j even
    jm_o, jp_o = slice(1, k, 2), slice(3, k + 2, 2)      # j, j+2 for j odd

    with tile_mod.TileContext(nc) as tc:
        with ExitStack() as ctx:
            xpool = ctx.enter_context(tc.tile_pool(name="xin", bufs=4))
            mpool = ctx.enter_context(tc.tile_pool(name="mid", bufs=2))
            opool = ctx.enter_context(tc.tile_pool(name="outp", bufs=4))

            def colspan(li):
                # X cols 0..wcl+1 <-> image cols lc0-1 .. lc0+wcl
                lo = 1 if li == 0 else 0
                hi = wcl + 1 if li == nl - 1 else wcl + 2
                return lo, hi, li * wcl - 1 + lo

            def load_main(li):
                lo, hi, dlo = colspan(li)
                ncol = hi - lo
                X = xpool.tile([p, k + 2, wcl + 2], db)
                # partition q holds image rows q*k - 1 .. q*k + k at jx
                # 0..k+1; halo rows load straight from HBM so every load is
                # independent (no SBUF->SBUF copies on the critical path).
                # middle partitions 1..p-2: all k+2 rows in one DMA
                nc.gpsimd.dma_start(
                    X[1 : p - 1, :, lo:hi],
                    bass.AP(
                        x,
                        (k - 1) * w + dlo,
                        [[k * w, p - 2], [w, k + 2], [1, ncol]],
                    ),
                )
                # partition 0: rows 0..k at jx 1..k+1
                nc.gpsimd.dma_start(
                    X[0:1, 1 : k + 2, lo:hi],
                    bass.AP(x, dlo, [[k * w, 1], [w, k + 1], [1, ncol]]),
                )
                # partition p-1: rows h-k-1..h-1 at jx 0..k
                nc.gpsimd.dma_start(
                    X[p - 1 : p, 0 : k + 1, lo:hi],
                    bass.AP(
                        x,
                        (h - k - 1) * w + dlo,
                        [[k * w, 1], [w, k + 1], [1, ncol]],
                    ),
                )
                # reflect row h := row h-2 (tiny DMA; compute engines
                # cannot address a lone partition p-1)
                nc.gpsimd.dma_start(
                    X[p - 1 : p, k + 1 : k + 2, lo:hi],
                    x[h - 2 : h - 1, dlo : dlo + ncol].unsqueeze(0),
                )
                return X

            def stage_halo(li, X):
                lo, hi, _ = colspan(li)
                # reflect row -1 := row 1 (= jx 2 of partition 0;
                # in-partition copy, legal at partition start 0)
                nc.vector.tensor_copy(X[0:1, 0:1, lo:hi], X[0:1, 2:3, lo:hi])
                # reflect cols: image col -1 := col 1 ; col w := col w-2
                if li == 0:
                    nc.vector.tensor_copy(X[:, :, 0:1], X[:, :, 2:3])
                if li == nl - 1:
                    nc.vector.tensor_copy(
                        X[:, :, wcl + 1 : wcl + 2], X[:, :, wcl - 1 : wcl]
                    )
                # prescale in place: X now holds 0.25*x (bf16, 4x mode)
                nc.vector.tensor_scalar_mul(X[:], X[:], 0.25)

            def stage_compute(li, X, tile_idx):
                lc0 = li * wcl
                for ci in range(ncc):
                    off = ci * wcc          # X col off+1 <-> img col cc0
                    cc0 = lc0 + off

                    Hq = mpool.tile([p, k + 2, wcc], db, tag="hq")
                    nc.vector.tensor_add(
                        Hq[:],
                        X[:, :, off : off + wcc],
                        X[:, :, off + 2 : off + wcc + 2],
                    )
                    Vq = mpool.tile([p, k, wcc], db, tag="vq")
                    nc.vector.tensor_add(
                        Vq[:],
                        X[:, 0:k, off + 1 : off + wcc + 1],
                        X[:, 2 : k + 2, off + 1 : off + wcc + 1],
                    )

                    O = opool.tile([p, 3, k, wcc], dt, tag="o")

                    ce, co = slice(0, wcc, 2), slice(1, wcc, 2)
                    cxe = slice(off + 1, off + wcc + 1, 2)
                    cxo = slice(off + 2, off + wcc + 2, 2)

                    act = nc.scalar.activation
                    tt = nc.vector.tensor_add
                    # R
                    act(O[:, 0, je, ce], X[:, jxe, cxe], Copy, scale=4.0)
                    act(O[:, 0, je, co], Hq[:, jxe, co], Copy, scale=2.0)
                    act(O[:, 0, jo, ce], Vq[:, jo, ce], Copy, scale=2.0)
                    tt(O[:, 0, jo, co], Hq[:, jm_o, co], Hq[:, jp_o, co])
                    # G
                    tt(O[:, 1, je, ce], Vq[:, je, ce], Hq[:, jxe, ce])
                    act(O[:, 1, je, co], X[:, jxe, cxo], Copy, scale=4.0)
                    act(O[:, 1, jo, ce], X[:, jxo, cxe], Copy, scale=4.0)
                    tt(O[:, 1, jo, co], Vq[:, jo, co], Hq[:, jxo, co])
                    # B
                    tt(O[:, 2, je, ce], Hq[:, jm_e, ce], Hq[:, jp_e, ce])
                    act(O[:, 2, je, co], Vq[:, je, co], Copy, scale=2.0)
                    act(O[:, 2, jo, ce], Hq[:, jxo, ce], Copy, scale=2.0)
                    act(O[:, 2, jo, co], X[:, jxo, cxo], Copy, scale=4.0)

                    # stores: f32, alternate the two HWDGE rings
                    eng_a = nc.scalar if tile_idx % 2 == 0 else nc.sync
                    eng_b = nc.sync if tile_idx % 2 == 0 else nc.scalar
                    for ch, eng in ((0, eng_a), (1, eng_b), (2, eng_a)):
                        eng.dma_start(
                            outr[:, ch, :, cc0 : cc0 + wcc], O[:, ch]
                        )
                    tile_idx += 1

            # software-pipelined emission: keep main loads LA tiles ahead of
            # the halo/compute stages so the halo copies' wait on the main
            # load never stalls load issuance on the gpsimd queue.
            LA = 2
            xs = []
            for li in range(nl):
                xs.append(load_main(li))
                if li >= LA:
                    stage_halo(li - LA, xs[li - LA])
                    stage_compute(li - LA, xs[li - LA], (li - LA) * ncc)
                    xs[li - LA] = None
            for li in range(nl - LA, nl):
                stage_halo(li, xs[li])
                stage_compute(li, xs[li], li * ncc)
                xs[li] = None
    if bacc_mod is not None:
        nc.compile()
    return nc


def _get_nc():
    key = (H, W, K, WCL, WCC)
    if key not in _compiled:
        import concourse.bass as bass
        import concourse.tile as tile
        from concourse import bacc, mybir

        _compiled[key] = _build(bass, tile, mybir, H, W, K, WCL, WCC, bacc_mod=bacc)
    return _compiled[key]


def kernel(x: np.ndarray, kernels: np.ndarray | None = None) -> np.ndarray:
    """x: (8, 1, 2048, 2048) f32 -> (8, 3, 2048, 2048) f32."""
    from concourse.bass_utils import run_bass_kernel_spmd

    x = np.ascontiguousarray(np.asarray(x, dtype=np.float32))
    b = x.shape[0]
    assert x.shape == (b, 1, H, W) and b == N_CORES
    nc = _get_nc()
    in_maps = [{"x": x[i, 0]} for i in range(b)]
    res = run_bass_kernel_spmd(nc, in_maps, list(range(N_CORES)))
    return np.stack([res.results[i]["out"] for i in range(b)], axis=0)


# revision 12
# speedup vs baseline: 1.1321x; 1.1321x over previous
"""Debayer3x3 Trainium2 Bass kernel.

Reference computation (per image, H=W=2048, f32):
  xpad = reflect-pad(x, 1)
  cross = 0.25*(up+down+left+right), diag = 0.25*(4 diagonals),
  hz = 0.5*(left+right), vt = 0.5*(up+down)
  R = [[x, hz], [vt, diag]]  (2x2 row/col parity pattern)
  G = [[cross, x], [x, cross]]
  B = [[diag, vt], [hz, x]]

Strategy: pure data parallel, 1 image per NeuronCore (batch 8 / 8 cores).
Per core, tiles put K=16 consecutive image rows in each SBUF partition so all
vertical neighbor access is a free-dim shift; one band of 128 partitions
covers the whole image, so the halo re-read is only 2/K = 12.5%.

All intermediates are bf16 (output tolerance is 2e-2; bf16 rounding is
~6e-3 worst case here and every scale is a power of two, hence exact):
  - loads cast f32->bf16 in-flight on the SWDGE (gpsimd) path, halving both
    SBUF footprint and DVE time (bf16 tensor_tensor runs in 2x perf mode),
  - X is prescaled by 0.25 in place (bf16 tensor_scalar, 4x mode) so every
    final op needs only a free ACT scale or a plain add,
  - quadrant ops write an f32 output tile directly (engine output cast).

Work planes per compute chunk (X holds 0.25*x):
  Hq[jx,c] = X[jx,c] + X[jx,c+2]   (= 0.25*(left+right) at row jx, col c)
  Vq[j,c]  = X[j,c+1] + X[j+2,c+1] (= 0.25*(up+down) at out row j, col c)
Each output quadrant is one scaled ACT copy (hz=2*Hq, vt=2*Vq, x=4*X) on the
scalar engine or one strided bf16 tensor_add (diag=Hq[j]+Hq[j+2],
cross=Vq[j]+Hq[j+1]) on the vector engine, written into an interleaved
[p, 3ch, K, WCC] f32 output tile.

DMA: loads (incl. halos) ride the SWDGE ring (nc.gpsimd, needed for the
dtype cast); f32 stores alternate between the two HWDGE rings (nc.scalar /
nc.sync) so three DMA paths drain concurrently.
"""

from contextlib import ExitStack

import numpy as np

H, W = 2048, 2048
K = 16           # image rows per partition (one 128-partition band = image)
WCL = 256        # load chunk width
WCC = 256        # compute/store chunk width
N_CORES = 8

_compiled = {}


def _build(nc_mod, tile_mod, mybir, h, w, k, wcl, wcc, bacc_mod=None):
    """Emit the debayer program for one core into a fresh Bass object."""
    bass = nc_mod
    p = 128
    nl = w // wcl          # load chunks per row-band
    ncc = wcl // wcc       # compute chunks per load chunk
    assert h == p * k and w == nl * wcl and wcl == ncc * wcc
    assert k % 2 == 0 and wcc % 2 == 0
    dt = mybir.dt.float32
    db = mybir.dt.bfloat16
    Copy = mybir.ActivationFunctionType.Copy

    nc = bass.Bass() if bacc_mod is None else bacc_mod.Bacc("TRN2")
    x = nc.dram_tensor("x", [h, w], dt, kind="ExternalInput")
    out = nc.dram_tensor("out", [3, h, w], dt, kind="ExternalOutput")
    outr = out.rearrange("c (p k) w -> p c k w", p=p, k=k)

    # row slices into the output tile (j) and into X/Hq (jx = j+1 is the same
    # image row; X/Hq rows 0..k+1 carry the halo)
    je, jo = slice(0, k, 2), slice(1, k, 2)
    jxe, jxo = slice(1, k + 1, 2), slice(2, k + 2, 2)
    jm_e, jp_e = slice(0, k, 2), slice(2, k + 2, 2)      # j, j+2 for j even
    jm_o, jp_o = slice(1, k, 2), slice(3, k + 2, 2)      # j, j+2 for j odd

    with tile_mod.TileContext(nc) as tc:
        with ExitStack() as ctx:
            xpool = ctx.enter_context(tc.tile_pool(name="xin", bufs=4))
            mpool = ctx.enter_context(tc.tile_pool(name="mid", bufs=2))
            opool = ctx.enter_context(tc.tile_pool(name="outp", bufs=2))

            def colspan(li):
                # X cols 0..wcl+1 <-> image cols lc0-1 .. lc0+wcl
                lo = 1 if li == 0 else 0
                hi = wcl + 1 if li == nl - 1 else wcl + 2
                return lo, hi, li * wcl - 1 + lo

            def load_main(li):
                lo, hi, dlo = colspan(li)
                ncol = hi - lo
                X = xpool.tile([p, k + 2, wcl + 2], db)
                # partition q holds image rows q*k - 1 .. q*k + k at jx
                # 0..k+1; halo rows load straight from HBM so every load is
                # independent (no SBUF->SBUF copies on the critical path).
                # middle partitions 1..p-2: all k+2 rows in one DMA
                nc.gpsimd.dma_start(
                    X[1 : p - 1, :, lo:hi],
                    bass.AP(
                        x,
                        (k - 1) * w + dlo,
                        [[k * w, p - 2], [w, k + 2], [1, ncol]],
                    ),
                )
                # partition 0: rows 0..k at jx 1..k+1
                nc.gpsimd.dma_start(
                    X[0:1, 1 : k + 2, lo:hi],
                    bass.AP(x, dlo, [[k * w, 1], [w, k + 1], [1, ncol]]),
                )
                # partition p-1: rows h-k-1..h-1 at jx 0..k
                nc.gpsimd.dma_start(
                    X[p - 1 : p, 0 : k + 1, lo:hi],
                    bass.AP(
                        x,
                        (h - k - 1) * w + dlo,
                        [[k * w, 1], [w, k + 1], [1, ncol]],
                    ),
                )
                # reflect row h := row h-2 (tiny DMA; compute engines
                # cannot address a lone partition p-1)
                nc.gpsimd.dma_start(
                    X[p - 1 : p, k + 1 : k + 2, lo:hi],
                    x[h - 2 : h - 1, dlo : dlo + ncol].unsqueeze(0),
                )
                return X

            def stage_halo(li, X):
                lo, hi, _ = colspan(li)
                # reflect row -1 := row 1 (= jx 2 of partition 0;
                # in-partition copy, legal at partition start 0)
                nc.vector.tensor_copy(X[0:1, 0:1, lo:hi], X[0:1, 2:3, lo:hi])
                # reflect cols: image col -1 := col 1 ; col w := col w-2
                if li == 0:
                    nc.vector.tensor_copy(X[:, :, 0:1], X[:, :, 2:3])
                if li == nl - 1:
                    nc.vector.tensor_copy(
                        X[:, :, wcl + 1 : wcl + 2], X[:, :, wcl - 1 : wcl]
                    )
                # prescale in place: X now holds 0.25*x (bf16, 4x mode)
                nc.vector.tensor_scalar_mul(X[:], X[:], 0.25)

            def stage_compute(li, X, tile_idx):
                lc0 = li * wcl
                for ci in range(ncc):
                    off = ci * wcc          # X col off+1 <-> img col cc0
                    cc0 = lc0 + off

                    Hq = mpool.tile([p, k + 2, wcc], db, tag="hq")
                    nc.vector.tensor_add(
                        Hq[:],
                        X[:, :, off : off + wcc],
                        X[:, :, off + 2 : off + wcc + 2],
                    )
                    Vq = mpool.tile([p, k, wcc], db, tag="vq")
                    nc.vector.tensor_add(
                        Vq[:],
                        X[:, 0:k, off + 1 : off + wcc + 1],
                        X[:, 2 : k + 2, off + 1 : off + wcc + 1],
                    )

                    O = opool.tile([p, 3, k, wcc], dt, tag="o")

                    ce, co = slice(0, wcc, 2), slice(1, wcc, 2)
                    cxe = slice(off + 1, off + wcc + 1, 2)
                    cxo = slice(off + 2, off + wcc + 2, 2)

                    act = nc.scalar.activation
                    tt = nc.vector.tensor_add
                    # R
                    act(O[:, 0, je, ce], X[:, jxe, cxe], Copy, scale=4.0)
                    act(O[:, 0, je, co], Hq[:, jxe, co], Copy, scale=2.0)
                    act(O[:, 0, jo, ce], Vq[:, jo, ce], Copy, scale=2.0)
                    tt(O[:, 0, jo, co], Hq[:, jm_o, co], Hq[:, jp_o, co])
                    # G
                    tt(O[:, 1, je, ce], Vq[:, je, ce], Hq[:, jxe, ce])
                    act(O[:, 1, je, co], X[:, jxe, cxo], Copy, scale=4.0)
                    act(O[:, 1, jo, ce], X[:, jxo, cxe], Copy, scale=4.0)
                    tt(O[:, 1, jo, co], Vq[:, jo, co], Hq[:, jxo, co])
                    # B
                    tt(O[:, 2, je, ce], Hq[:, jm_e, ce], Hq[:, jp_e, ce])
                    act(O[:, 2, je, co], Vq[:, je, co], Copy, scale=2.0)
                    act(O[:, 2, jo, ce], Hq[:, jxo, ce], Copy, scale=2.0)
                    act(O[:, 2, jo, co], X[:, jxo, cxo], Copy, scale=4.0)

                    # stores: f32, alternate the two HWDGE rings
                    eng_a = nc.scalar if tile_idx % 2 == 0 else nc.sync
                    eng_b = nc.sync if tile_idx % 2 == 0 else nc.scalar
                    for ch, eng in ((0, eng_a), (1, eng_b), (2, eng_a)):
                        eng.dma_start(
                            outr[:, ch, :, cc0 : cc0 + wcc], O[:, ch]
                        )
                    tile_idx += 1

            # software-pipelined emission: keep main loads LA tiles ahead of
            # the halo/compute stages so the halo copies' wait on the main
            # load never stalls load issuance on the gpsimd queue.
            LA = 2
            xs = []
            for li in range(nl):
                xs.append(load_main(li))
                if li >= LA:
                    stage_halo(li - LA, xs[li - LA])
                    stage_compute(li - LA, xs[li - LA], (li - LA) * ncc)
                    xs[li - LA] = None
            for li in range(nl - LA, nl):
                stage_halo(li, xs[li])
                stage_compute(li, xs[li], li * ncc)
                xs[li] = None
    if bacc_mod is not None:
        nc.compile()
    return nc


def _get_nc():
    key = (H, W, K, WCL, WCC)
    if key not in _compiled:
        import concourse.bass as bass
        import concourse.tile as tile
        from concourse import bacc, mybir

        _compiled[key] = _build(bass, tile, mybir, H, W, K, WCL, WCC, bacc_mod=bacc)
    return _compiled[key]


def kernel(x: np.ndarray, kernels: np.ndarray | None = None) -> np.ndarray:
    """x: (8, 1, 2048, 2048) f32 -> (8, 3, 2048, 2048) f32."""
    from concourse.bass_utils import run_bass_kernel_spmd

    x = np.ascontiguousarray(np.asarray(x, dtype=np.float32))
    b = x.shape[0]
    assert x.shape == (b, 1, H, W) and b == N_CORES
    nc = _get_nc()
    in_maps = [{"x": x[i, 0]} for i in range(b)]
    res = run_bass_kernel_spmd(nc, in_maps, list(range(N_CORES)))
    return np.stack([res.results[i]["out"] for i in range(b)], axis=0)
